# revision 12
# baseline (speedup 1.0000x reference)
"""AffNetR TRN2 kernel: out[u,i] = ((max_h cos(Z[h,u,:], X[i,:])) + 1) / 2, ^beta.

Sharding: data-parallel over users (U=8192) across 8 NeuronCores; X replicated.
Each core computes a [1024, 8192] slice of the output.

v3: bf16 operands + engine-balanced evacuation.

Inputs arrive pre-transposed and pre-cast to bf16 ([E=128, *]). Norms via
accumulating selector matmuls: chunk c's column-sum-of-squares uses a one-hot
column stationary so the [1,512] result lands on PSUM partition c, all chunks
accumulating into one [16,512] (X) / [8,512] (Z) PSUM tile — no HBM bounce.
One ACT Rsqrt (scale=4 folds the /2 affine into the X side) emits bf16 scales.
Per chunk: a selector matmul broadcasts the scale row to 128 partitions, ACT
evacuates it to bf16, and a DVE all-bf16 tensor-tensor multiply (2x_1p mode)
writes the normalized operand.

Main loop per (u-tile 128, i-tile 512): 4 bf16 matmuls (one per head, order
h0,h2,h1,h3) into one [128,2048] PSUM tile (4 banks, double-buffered). ACT
evacuates banks 0-1 in one wide [128,1024] op (+0.5 bias) to bf16; DVE folds
banks 2-3 with one wide scalar_tensor_tensor ((p+0.5) max c) to bf16; GpSimd
does the final SBUF-only bf16 max into the fp32 output stage; Sync issues the
store DMAs. X-chunk normalization for chunks 1-15 interleaves into ut=0.

A post-Tile pass splits excess semaphore waits onto inserted NoOps.
"""

import numpy as np

import concourse.bass as bass
import concourse.mybir as mybir
import concourse.tile as tile
from concourse.bass_utils import run_bass_kernel_spmd

F32 = mybir.dt.float32
F32R = mybir.dt.float32r
BF16 = mybir.dt.bfloat16

H = 4
U = 8192
E = 128
I = 8192
NCORES = 8
USH = U // NCORES          # 1024 users per core
UT = USH // 128            # 8 u-tiles
IT = I // 512              # 16 i-tiles
NXC = I // 512             # 16 x chunks of 512
NZC = (H * USH) // 512     # 8 z chunks of 512

_cache = {}


def _legalize_waits(nc, max_waits=1):
    """Hoist excess sem waits onto same-engine NoOps (1-wait ISA structs)."""
    cnt = 0
    for f in nc.m.functions:
        for blk in f.blocks:
            insts = blk.instructions
            out = []
            changed = False
            for inst in insts:
                si = inst.sync_info
                waits = list(si.on_wait) if si is not None and si.on_wait else []
                if len(waits) > max_waits and inst.engine is not None:
                    keep = waits[-max_waits:]
                    for w in waits[:-max_waits]:
                        nop = mybir.InstNoOp(name=f"wlg-{cnt}", ins=[], outs=[])
                        cnt += 1
                        nop.engine = inst.engine
                        nop.sync_info = mybir.SyncInfo(on_wait=[w], on_update=[])
                        out.append(nop)
                    upd = list(si.on_update) if si.on_update else []
                    inst.sync_info = mybir.SyncInfo(on_wait=keep, on_update=upd)
                    changed = True
                out.append(inst)
            if changed:
                blk.instructions = out
    return cnt


def _build():
    nc = bass.Bass()
    xt_d = nc.dram_tensor("xt", [E, I], BF16, kind="ExternalInput")
    zt_d = nc.dram_tensor("zt", [E, H * USH], BF16, kind="ExternalInput")
    # colsum stationaries: slice c is a [128, k] matrix whose column c is ones
    selx_d = nc.dram_tensor("selx", [E, NXC * NXC], BF16, kind="ExternalInput")
    selz_d = nc.dram_tensor("selz", [E, NZC * NZC], BF16, kind="ExternalInput")
    # broadcast stationaries: slice c is a [k, 128] matrix whose row c is ones
    selbx_d = nc.dram_tensor("selbx", [NXC, NXC * 128], F32R, kind="ExternalInput")
    selbz_d = nc.dram_tensor("selbz", [NZC, NZC * 128], F32R, kind="ExternalInput")
    out_d = nc.dram_tensor("out", [USH, I], BF16, kind="ExternalOutput")
    out_v = out_d[:].rearrange("(uo p) i -> p uo i", p=128)

    S = mybir.ActivationFunctionType
    A = mybir.AluOpType

    with tile.TileContext(nc) as tc:
        with tc.tile_pool(name="big", bufs=1) as big:
            xt_sb = big.tile([E, I], BF16, tag="xt_sb")
            zt_sb = big.tile([E, H * USH], BF16, tag="zt_sb")
            xtn = big.tile([E, I], BF16, tag="xtn")
            ztn = big.tile([E, H * USH], BF16, tag="ztn")
            selx = big.tile([E, NXC * NXC], BF16, tag="selx")
            selz = big.tile([E, NZC * NZC], BF16, tag="selz")
            selbx = big.tile([NXC, NXC * 128], F32R, tag="selbx")
            selbz = big.tile([NZC, NZC * 128], F32R, tag="selbz")
            rx05 = big.tile([NXC, 512], F32, tag="rx05")
            rz1 = big.tile([NZC, 512], F32, tag="rz1")
            half1 = big.tile([128, 1], F32, tag="half1")

            # input DMAs: z first (gates everything), then x, then sels
            nc.sync.dma_start(zt_sb[:, 0:2048], zt_d[:, 0:2048])
            nc.sync.dma_start(zt_sb[:, 2048:4096], zt_d[:, 2048:4096])
            for q in range(4):
                s = slice(q * 2048, (q + 1) * 2048)
                nc.sync.dma_start(xt_sb[:, s], xt_d[:, s])
            nc.sync.dma_start(selz, selz_d[:])
            nc.sync.dma_start(selx, selx_d[:])
            nc.sync.dma_start(selbz, selbz_d[:])
            nc.sync.dma_start(selbx, selbx_d[:])

            nc.vector.memset(half1, 0.5)

            sq_ctx = tc.tile_pool(name="sq", bufs=4)
            sq_pool = sq_ctx.__enter__()
            rep_ctx = tc.tile_pool(name="repb", bufs=4)
            rep_pool = rep_ctx.__enter__()
            pcs_ctx = tc.tile_pool(name="pcs", bufs=2, space="PSUM")
            pcs = pcs_ctx.__enter__()
            prep_ctx = tc.tile_pool(name="prep", bufs=2, space="PSUM")
            prep = prep_ctx.__enter__()

            def colsums(src, sel, nchunks, npar, tag):
                """sum of squares per column, accumulated into [npar,512] PSUM
                via one-hot-column stationaries (chunk c lands on partition c)."""
                g = pcs.tile([npar, 512], F32, tag=tag)
                for c in range(nchunks):
                    s = slice(c * 512, (c + 1) * 512)
                    sq = sq_pool.tile([E, 512], BF16, tag="sq")
                    nc.vector.tensor_tensor(sq, src[:, s], src[:, s], A.mult)
                    nc.tensor.matmul(
                        g,
                        sel[:, c * npar : (c + 1) * npar],
                        sq,
                        start=(c == 0),
                        stop=(c == nchunks - 1),
                    )
                return g

            def normalize(src, dst, selb, scales, nchunks, npar, lim=None):
                """dst chunk = src chunk * broadcast(scales row c)."""
                for c in range(nchunks if lim is None else lim):
                    norm_chunk(src, dst, selb, scales, npar, c)

            def norm_chunk(src, dst, selb, scales, npar, c):
                s = slice(c * 512, (c + 1) * 512)
                rep = prep.tile([128, 512], F32, tag="rep")
                nc.tensor.matmul(
                    rep,
                    selb[0:npar, c * 128 : (c + 1) * 128],
                    scales,
                    start=True,
                    stop=True,
                )
                repb = rep_pool.tile([128, 512], BF16, tag="repb")
                nc.scalar.activation(repb, rep, S.Identity)
                nc.vector.tensor_tensor(dst[:, s], src[:, s], repb, A.mult)

            gz = colsums(zt_sb, selz, NZC, NZC, "gz")
            # rz1r = 1/sqrt(ssz)  [f32r, partitions 0:8]
            sz = big.tile([NZC, 512], F32, tag="sz")
            nc.scalar.activation(sz, gz, S.Sqrt)
            nc.vector.reciprocal(rz1, sz)
            rz1r = big.tile([NZC, 512], F32R, tag="rz1r")
            nc.scalar.copy(rz1r, rz1)
            gx = colsums(xt_sb, selx, NXC, NXC, "gx")
            # rx05r = 0.5/sqrt(ssx) = 1/sqrt(4*ssx)  [f32r, partitions 0:16]
            sx = big.tile([NXC, 512], F32, tag="sx")
            nc.scalar.activation(sx, gx, S.Sqrt, scale=4.0)
            nc.vector.reciprocal(rx05, sx)
            rx05r = big.tile([NXC, 512], F32R, tag="rx05r")
            nc.scalar.copy(rx05r, rx05)

            normalize(zt_sb, ztn, selbz, rz1r, NZC, NZC)
            normalize(xt_sb, xtn, selbx, rx05r, NXC, NXC)

            prep_ctx.__exit__(None, None, None)
            pcs_ctx.__exit__(None, None, None)
            rep_ctx.__exit__(None, None, None)
            sq_ctx.__exit__(None, None, None)

            # ---------- main loop ----------
            with (
                tc.tile_pool(name="work", bufs=3) as work,
                tc.tile_pool(name="ost", bufs=2) as ost,
                tc.tile_pool(name="pmm", bufs=2, space="PSUM") as pmm,
            ):
                # PSUM slice layout per tile: [h0 | h2 | h1 | h3]
                order = (0, 2, 1, 3)
                for ut in range(UT):
                    lhs = [
                        ztn[:, h * USH + ut * 128 : h * USH + (ut + 1) * 128]
                        for h in range(H)
                    ]
                    for it in range(IT):
                        rhs = xtn[:, it * 512 : (it + 1) * 512]
                        P = pmm.tile([128, 2048], F32, tag="P")
                        for k, h in enumerate(order):
                            nc.tensor.matmul(
                                P[:, k * 512 : (k + 1) * 512],
                                lhs[h],
                                rhs,
                                start=True,
                                stop=True,
                            )
                        c = work.tile([128, 1024], BF16, tag="c")
                        nc.scalar.activation(
                            c, P[:, 0:1024], S.Identity, bias=half1, scale=1.0
                        )
                        m = work.tile([128, 1024], BF16, tag="m")
                        nc.vector.scalar_tensor_tensor(
                            m, P[:, 1024:2048], 0.5, c, op0=A.add, op1=A.max
                        )
                        if it % 4 == 0:
                            ostage = ost.tile([128, 2048], BF16, tag="ostage")
                        nc.vector.tensor_tensor(
                            ostage[:, (it % 4) * 512 : (it % 4 + 1) * 512],
                            m[:, 0:512],
                            m[:, 512:1024],
                            A.max,
                        )
                        if ut == UT - 1 and it >= 12:
                            j = it % 4
                            nc.sync.dma_start(
                                out_v[:, ut, (12 + j) * 512 : (13 + j) * 512],
                                ostage[:, j * 512 : (j + 1) * 512],
                            )
                        elif it % 4 == 3:
                            ig = it // 4
                            nc.sync.dma_start(
                                out_v[:, ut, ig * 2048 : (ig + 1) * 2048],
                                ostage,
                            )

    _legalize_waits(nc)
    return nc


def _sel_hosts():
    import ml_dtypes

    bf = ml_dtypes.bfloat16
    selx = np.zeros((E, NXC * NXC), dtype=bf)
    for c in range(NXC):
        selx[:, c * NXC + c] = 1.0
    selz = np.zeros((E, NZC * NZC), dtype=bf)
    for c in range(NZC):
        selz[:, c * NZC + c] = 1.0
    selbx = np.zeros((NXC, NXC * 128), dtype=np.float32)
    for c in range(NXC):
        selbx[c, c * 128 : (c + 1) * 128] = 1.0
    selbz = np.zeros((NZC, NZC * 128), dtype=np.float32)
    for c in range(NZC):
        selbz[c, c * 128 : (c + 1) * 128] = 1.0
    return selx, selz, selbx, selbz


def _in_maps(X, Z):
    import ml_dtypes

    bf = ml_dtypes.bfloat16
    X = np.asarray(X, dtype=np.float32)
    Z = np.asarray(Z, dtype=np.float32)
    xt = np.ascontiguousarray(X.T).astype(bf)            # [128, 8192]
    selx, selz, selbx, selbz = _sel_hosts()
    in_maps = []
    for c in range(NCORES):
        zs = Z[:, c * USH : (c + 1) * USH, :]            # [4, 1024, 128]
        zt = np.ascontiguousarray(
            zs.transpose(2, 0, 1).reshape(E, H * USH)
        ).astype(bf)                                     # [128, 4096]
        in_maps.append(
            {
                "xt": xt,
                "zt": zt,
                "selx": selx,
                "selz": selz,
                "selbx": selbx,
                "selbz": selbz,
            }
        )
    return in_maps


def kernel(X, Z, beta):
    in_maps = _in_maps(X, Z)
    if "nc" not in _cache:
        _cache["nc"] = _build()
    res = run_bass_kernel_spmd(_cache["nc"], in_maps, list(range(NCORES))).results
    out = np.concatenate([r["out"] for r in res], axis=0).astype(np.float32)

    b = float(np.asarray(beta))
    if b != 1.0:
        out = np.power(out, b).astype(np.float32)
    return out
